# revision 1
# baseline (speedup 1.0000x reference)
"""Entropic Sinkhorn loss kernel for Trainium2 (8 NeuronCores, SPMD).

Math (reference): loss = (sinkhorn(1 - img@txt.T) + sinkhorn((1 - img@txt.T).T)) / 2
with K = exp(-(M)/0.01) = exp(100*S - 100), S = img@txt.T, 5 Sinkhorn iterations,
then P = u * K * v, loss_half = -mean(log_softmax(P)[i, i]).

Device algorithm (per core, rows sharded B/8 = 1024):
  build: S row-shard GEMM (bf16) -> K = exp(100S - 100) (bf16) -> HBM, plus
         PE-transposed copy KT -> HBM; col-matvec of K with 1/n gives KtU_1.
  pass t (1..5), one pass over KT then K, all four matvecs on PE:
    row-matvecs (lhsT = KT blocks):  u_t = 1/(K v_t),  v'_t = b/(K u'_{t-1})
    col-matvecs (lhsT = K blocks):   KtU_{t+1} = K^T u_t -> v_{t+1} = b/AR(.)
                                     K^T v'_t -> u'_t = 1/AR(.)
    (AR = 64KB AllReduce across the 8 cores)
  loss: stream KT once; e1 = exp(v5_c * K[r,c] * u5_r) column-summed on PE
        (row-LSE of P1), e2 = exp(u'5_c * K[r,c] * v'5_r) row-summed via ACT
        accumulate (row-LSE of P2, needs AR); diagonal terms from host-provided
        K-diagonal; final scalar assembled identically on every core.
"""

import numpy as np
import ml_dtypes

import concourse.bass as bass
import concourse.bacc as bacc
import concourse.tile as tile
import concourse.mybir as mybir
from concourse import bass_utils
from concourse.bass import ds
from concourse.masks import make_identity

B = 8192
D = 256
P = 128
NCORES = 8
R = B // NCORES            # 1024 rows per core
RT = R // P                # 8 row tiles per core
CT = B // P                # 64 col tiles
NCH = B // 512             # 16 GEMM chunks of 512
N_ITER = 5
INV_REG = 100.0
BVAL = 1.0 / B

BF16 = mybir.dt.bfloat16
F32 = mybir.dt.float32
Exp = mybir.ActivationFunctionType.Exp
Log = mybir.ActivationFunctionType.Ln
ADD = mybir.AluOpType.add
MULT = mybir.AluOpType.mult


def _build_bass(b=None, phases=3, loss_stop=0, ncores=None):
    global B, R, RT, CT, NCH, BVAL, NCORES
    if ncores is not None:
        NCORES = ncores
    if b is not None:
        B = b
        R = B // NCORES
        RT = max(R // P, 1)
        CT = B // P
        NCH = max(B // 512, 1)
        BVAL = 1.0 / B
    nc = bacc.Bacc("TRN2", target_bir_lowering=False, debug=False,
                   num_devices=NCORES)

    imgT = nc.dram_tensor("imgT", [P, 2, R], BF16, kind="ExternalInput")
    txtT = nc.dram_tensor("txtT", [P, 2, B], BF16, kind="ExternalInput")
    kdiag_in = nc.dram_tensor("kdiag", [P, RT], F32, kind="ExternalInput")
    dsel_in = nc.dram_tensor("dsel", [P, RT, CT], F32, kind="ExternalInput")
    out_loss = nc.dram_tensor("out_loss", [1, 1], F32, kind="ExternalOutput")

    K_hbm = nc.dram_tensor("K_hbm", [R, B], BF16)
    KT_hbm = nc.dram_tensor("KT_hbm", [B, R], BF16)
    RG = [list(range(NCORES))]
    ar_in = [nc.dram_tensor(f"ar_in{t}", [P, 2 * CT], F32) for t in range(6)]
    ar_out = [nc.dram_tensor(f"ar_out{t}", [P, 2 * CT], F32, addr_space="Shared")
              for t in range(6)]
    ar2_in = nc.dram_tensor("ar2_in", [P, CT + 2], F32)
    ar2_out = nc.dram_tensor("ar2_out", [P, CT + 2], F32, addr_space="Shared")

    def allreduce(t, src_ap, dst_ap):
        if NCORES == 1:
            nc.gpsimd.dma_start(out=dst_ap, in_=src_ap)
            return
        nc.gpsimd.dma_start(out=ar_in[t][:], in_=src_ap)
        nc.gpsimd.collective_compute(
            "AllReduce", ADD, replica_groups=RG,
            ins=[ar_in[t][:]], outs=[ar_out[t][:]])
        nc.gpsimd.dma_start(out=dst_ap, in_=ar_out[t][:])

    with tile.TileContext(nc) as tc:
        with tc.tile_pool(name="persist", bufs=1) as pp:
            identity_bf = pp.tile([P, P], BF16, tag="identity")
            make_identity(nc, identity_bf[:])
            ones_bf = pp.tile([P, 1], BF16, tag="ones_bf")
            nc.vector.memset(ones_bf[:], 1.0)
            ones_f = pp.tile([P, 1], F32, tag="ones_f")
            nc.vector.memset(ones_f[:], 1.0)
            ones_row = pp.tile([1, P], BF16, tag="ones_row")
            nc.vector.memset(ones_row[:], 1.0)
            biasm100 = pp.tile([P, 1], F32, tag="biasm100")
            nc.vector.memset(biasm100[:], -INV_REG)
            kdiag_f = pp.tile([P, RT], F32, tag="kdiag")
            nc.sync.dma_start(out=kdiag_f[:], in_=kdiag_in[:])
            dsel_bf = pp.tile([P, RT, CT], F32, tag="dsel")
            nc.sync.dma_start(out=dsel_bf[:], in_=dsel_in[:])

            # per-pass persistent vectors
            # vt[t]: [p, 2*cb+vec] f32; vec0 = v_{t+1} (loss1), vec1 = u'_t (loss2)
            vt_f = [pp.tile([P, 2 * CT], F32, tag=f"vtf{t}", name=f"vtf{t}")
                    for t in range(6)]
            vt_bf = [pp.tile([P, 2 * CT], BF16, tag=f"vtbf{t}", name=f"vtbf{t}")
                     for t in range(6)]
            # uvrow (pass5): [p, 2*rb+vec] f32; vec0 = u_5, vec1 = v'_5
            uvrow5_f = pp.tile([P, 2 * RT], F32, tag="uvrow5f")
            uvrow5_bf = pp.tile([P, 2 * RT], BF16, tag="uvrow5bf")
            cse2_f = pp.tile([P, CT], F32, tag="cse2")

            # ---------------- build phase ----------------
            with tc.tile_pool(name="bfeat", bufs=1) as pf, \
                 tc.tile_pool(name="bk", bufs=2) as pk, \
                 tc.tile_pool(name="bkt", bufs=3) as pkt, \
                 tc.tile_pool(name="bps", bufs=2, space="PSUM") as pps, \
                 tc.tile_pool(name="bpt", bufs=2, space="PSUM") as ppt, \
                 tc.tile_pool(name="bpc", bufs=1, space="PSUM") as ppc:
                imgT_sb = pf.tile([P, 2, R], BF16, tag="imgT")
                txtT_sb = pf.tile([P, 2, B], BF16, tag="txtT")
                nc.sync.dma_start(out=imgT_sb[:], in_=imgT[:])
                nc.sync.dma_start(out=txtT_sb[:], in_=txtT[:])
                invn_bf = pf.tile([P, 2], BF16, tag="invn")
                nc.vector.memset(invn_bf[:], BVAL)

                psum_cs = ppc.tile([P, 2 * CT], F32, tag="psum_cs")
                for ri in range(RT):
                    ktile = pk.tile([P, B], BF16, tag="ktile")
                    for cj in range(NCH):
                        psum_s = pps.tile([P, 512], F32, tag="psum_s")
                        for dhi in range(2):
                            nc.tensor.matmul(
                                psum_s[:],
                                lhsT=imgT_sb[:, dhi, ri * P:(ri + 1) * P],
                                rhs=txtT_sb[:, dhi, cj * 512:(cj + 1) * 512],
                                start=(dhi == 0), stop=(dhi == 1))
                        nc.scalar.activation(
                            out=ktile[:, cj * 512:(cj + 1) * 512],
                            in_=psum_s[:], func=Exp,
                            scale=INV_REG, bias=biasm100[:])
                    nc.sync.dma_start(out=K_hbm[ri * P:(ri + 1) * P, :],
                                      in_=ktile[:])
                    # col-matvec with constant u0 = 1/n -> KtU_1 accumulation
                    for cb in range(CT):
                        nc.tensor.matmul(
                            psum_cs[:, 2 * cb:2 * cb + 2],
                            lhsT=ktile[:, cb * P:(cb + 1) * P],
                            rhs=invn_bf[:],
                            start=(ri == 0 and cb == 0),
                            stop=(ri == RT - 1 and cb == CT - 1),
                            skip_group_check=True)
                    # transposes: groups of 8 col-blocks
                    for g in range(CT // 8):
                        psum_t = ppt.tile([P, 1024], BF16, tag="psum_t")
                        for j in range(8):
                            cb = g * 8 + j
                            nc.tensor.transpose(
                                psum_t[:, j * P:(j + 1) * P],
                                ktile[:, cb * P:(cb + 1) * P],
                                identity_bf[:])
                        ktstage = pkt.tile([P, 1024], BF16, tag="ktstage")
                        if g % 2 == 0:
                            nc.scalar.copy(out=ktstage[:], in_=psum_t[:])
                        else:
                            nc.vector.tensor_copy(ktstage[:], psum_t[:])
                        dst = KT_hbm[g * 1024:(g + 1) * 1024,
                                     ri * P:(ri + 1) * P]
                        dst = dst.rearrange("(j p) r -> p j r", p=P)
                        src = ktstage[:].rearrange("p (j r) -> p j r", j=8)
                        nc.sync.dma_start(out=dst, in_=src)

                # build epilogue: AR(KtU_1) -> v_1 = b/KtU_1 ; u'_0 = 1/n
                cs_sb = pf.tile([P, 2 * CT], F32, tag="cs_sb")
                nc.scalar.copy(out=cs_sb[:], in_=psum_cs[:])
                g_sb = pf.tile([P, 2 * CT], F32, tag="g_sb")
                allreduce(0, cs_sb[:], g_sb[:])
                rec = pf.tile([P, 2 * CT], F32, tag="rec")
                nc.vector.reciprocal(rec[:], g_sb[:])
                v0 = vt_f[0][:].rearrange("p (c v) -> p c v", v=2)
                r0 = rec[:].rearrange("p (c v) -> p c v", v=2)
                nc.scalar.mul(out=v0[:, :, 0], in_=r0[:, :, 0], mul=BVAL)
                nc.vector.memset(v0[:, :, 1], BVAL)
                nc.vector.tensor_copy(vt_bf[0][:], vt_f[0][:])

            if phases < 2:
                dbg = pp.tile([P, 1], F32, tag="dbg")
                nc.vector.tensor_reduce(dbg[:], vt_f[0][:],
                                        axis=mybir.AxisListType.X, op=ADD)
                nc.sync.dma_start(out=out_loss[:], in_=dbg[0:1, 0:1])

            # ---------------- sinkhorn passes ----------------
            with tc.tile_pool(name="skt", bufs=4) as pskt, \
                 tc.tile_pool(name="sk", bufs=2) as psk, \
                 tc.tile_pool(name="ssm", bufs=2) as psm, \
                 tc.tile_pool(name="spr", bufs=2, space="PSUM") as ppr, \
                 tc.tile_pool(name="spc", bufs=2, space="PSUM") as ppcol:
                for t in range(1, (N_ITER + 1) if phases >= 2 else 1):
                    # stage A: row-matvecs over KT tiles
                    psum_r = ppr.tile([P, 2 * RT], F32, tag="psum_r")
                    for ct in range(CT):
                        kt_t = pskt.tile([P, R], BF16, tag="kt_t")
                        nc.sync.dma_start(
                            out=kt_t[:], in_=KT_hbm[ct * P:(ct + 1) * P, :])
                        for rb in range(RT):
                            nc.tensor.matmul(
                                psum_r[:, 2 * rb:2 * rb + 2],
                                lhsT=kt_t[:, rb * P:(rb + 1) * P],
                                rhs=vt_bf[t - 1][:, 2 * ct:2 * ct + 2],
                                start=(ct == 0 and rb == 0),
                                stop=(ct == CT - 1 and rb == RT - 1),
                                skip_group_check=True)
                    # u_t = 1/(K v_t); v'_t = b/(K u'_{t-1})
                    if t == N_ITER:
                        uv_f, uv_bf = uvrow5_f, uvrow5_bf
                    else:
                        uv_f = psm.tile([P, 2 * RT], F32, tag="uv_f")
                        uv_bf = psm.tile([P, 2 * RT], BF16, tag="uv_bf")
                    rr = psm.tile([P, 2 * RT], F32, tag="rr")
                    nc.vector.reciprocal(rr[:], psum_r[:])
                    rrv = rr[:].rearrange("p (r v) -> p r v", v=2)
                    uvv = uv_f[:].rearrange("p (r v) -> p r v", v=2)
                    nc.scalar.copy(out=uvv[:, :, 0], in_=rrv[:, :, 0])
                    nc.scalar.mul(out=uvv[:, :, 1], in_=rrv[:, :, 1], mul=BVAL)
                    nc.vector.tensor_copy(uv_bf[:], uv_f[:])

                    # stage B: col-matvecs over K tiles
                    psum_c = ppcol.tile([P, 2 * CT], F32, tag="psum_c")
                    for ri in range(RT):
                        k_t = psk.tile([P, B], BF16, tag="k_t")
                        nc.sync.dma_start(
                            out=k_t[:], in_=K_hbm[ri * P:(ri + 1) * P, :])
                        for cb in range(CT):
                            nc.tensor.matmul(
                                psum_c[:, 2 * cb:2 * cb + 2],
                                lhsT=k_t[:, cb * P:(cb + 1) * P],
                                rhs=uv_bf[:, 2 * ri:2 * ri + 2],
                                start=(ri == 0 and cb == 0),
                                stop=(ri == RT - 1 and cb == CT - 1),
                                skip_group_check=True)
                    # epilogue: AR -> v_{t+1} = b/KtU ; u'_t = 1/(K^T v'_t)
                    cs2 = psm.tile([P, 2 * CT], F32, tag="cs2")
                    nc.scalar.copy(out=cs2[:], in_=psum_c[:])
                    gg = psm.tile([P, 2 * CT], F32, tag="gg")
                    allreduce(t, cs2[:], gg[:])
                    rec2 = psm.tile([P, 2 * CT], F32, tag="rec2")
                    nc.vector.reciprocal(rec2[:], gg[:])
                    vv = vt_f[t][:].rearrange("p (c v) -> p c v", v=2)
                    rv = rec2[:].rearrange("p (c v) -> p c v", v=2)
                    nc.scalar.mul(out=vv[:, :, 0], in_=rv[:, :, 0], mul=BVAL)
                    nc.scalar.copy(out=vv[:, :, 1], in_=rv[:, :, 1])
                    nc.vector.tensor_copy(vt_bf[t][:], vt_f[t][:])

            if phases == 2:
                dbg = pp.tile([P, 1], F32, tag="dbg")
                nc.vector.tensor_reduce(dbg[:], uvrow5_f[:],
                                        axis=mybir.AxisListType.X, op=ADD)
                nc.sync.dma_start(out=out_loss[:], in_=dbg[0:1, 0:1])

            # ---------------- loss phase ----------------
            if phases >= 3:
              with tc.tile_pool(name="lkt", bufs=4) as plkt, \
                   tc.tile_pool(name="lm", bufs=3) as plm, \
                   tc.tile_pool(name="lsm", bufs=1) as pls, \
                   tc.tile_pool(name="lp1", bufs=1, space="PSUM") as ppl1, \
                   tc.tile_pool(name="lpf", bufs=1, space="PSUM") as pplf, \
                   tc.tile_pool(name="lpb", bufs=1, space="PSUM") as pplb:
                  # replicate u5 / v'5 along partitions: [p, r] = x[r]
                  reps = []
                  for vec in range(2):
                      psum_fl = pplf.tile([1, R], F32, tag="psum_fl")
                      for rb in range(RT):
                          nc.tensor.matmul(
                              psum_fl[0:1, rb * P:(rb + 1) * P],
                              lhsT=uvrow5_bf[:, 2 * rb + vec:2 * rb + vec + 1],
                              rhs=identity_bf[:],
                              start=(rb % 4 == 0),
                              stop=(rb % 4 == 3 or rb == RT - 1),
                              skip_group_check=True)
                      flat_bf = pls.tile([1, R], BF16, tag=f"flat{vec}")
                      nc.scalar.copy(out=flat_bf[:], in_=psum_fl[:])
                      rep = pls.tile([P, R], BF16, tag=f"rep{vec}",
                                     name=f"rep{vec}")
                      bcch = min(512, R)
                      for h in range(R // bcch):
                          sl = slice(h * bcch, (h + 1) * bcch)
                          psum_bc = pplb.tile([P, bcch], F32, tag="psum_bc",
                                              bufs=2 if R <= 1024 else 1,
                                              name="psum_bc")
                          nc.tensor.matmul(
                              psum_bc[:],
                              lhsT=ones_row[:],
                              rhs=flat_bf[0:1, sl],
                              start=True, stop=True)
                          if h % 2 == 0:
                              nc.scalar.copy(out=rep[:, sl], in_=psum_bc[:])
                          else:
                              nc.vector.tensor_copy(rep[:, sl], psum_bc[:])
                      reps.append(rep)
                  u5_rep, vp5_rep = reps

                  # diagonal terms (host kdiag + mask-select of v5 / u'5 columns)
                  if loss_stop in (0, 2, 3):
                      v4v = vt_f[4][:].rearrange("p (c v) -> p c v", v=2)
                      v5v = vt_f[5][:].rearrange("p (c v) -> p c v", v=2)
                      uv5 = uvrow5_f[:].rearrange("p (r v) -> p r v", v=2)
                      v5d = pls.tile([P, RT], F32, tag="v5d")
                      up5d = pls.tile([P, RT], F32, tag="up5d")
                      selscr = pls.tile([P, CT], F32, tag="selscr")
                      for rb in range(RT):
                          nc.vector.tensor_mul(selscr[:], v4v[:, :, 0],
                                               dsel_bf[:, rb, :])
                          nc.vector.tensor_reduce(
                              v5d[:, rb:rb + 1], selscr[:],
                              axis=mybir.AxisListType.X, op=ADD)
                          nc.vector.tensor_mul(selscr[:], v5v[:, :, 1],
                                               dsel_bf[:, rb, :])
                          nc.vector.tensor_reduce(
                              up5d[:, rb:rb + 1], selscr[:],
                              axis=mybir.AxisListType.X, op=ADD)
                      pd1 = pls.tile([P, RT], F32, tag="pd1")
                      pd2 = pls.tile([P, RT], F32, tag="pd2")
                      nc.vector.tensor_mul(pd1[:], uv5[:, :, 0], kdiag_f[:])
                      nc.vector.tensor_mul(pd1[:], pd1[:], v5d[:])
                      nc.vector.tensor_mul(pd2[:], uv5[:, :, 1], kdiag_f[:])
                      nc.vector.tensor_mul(pd2[:], pd2[:], up5d[:])

                  if loss_stop == 2:
                      dbg = pp.tile([P, 1], F32, tag="dbg")
                      nc.vector.tensor_reduce(dbg[:], pd1[:], axis=mybir.AxisListType.X, op=ADD)
                      nc.sync.dma_start(out=out_loss[:], in_=dbg[0:1, 0:1])
                  if loss_stop in (0, 3):
                      # main streamed loop over KT
                      psum_l1 = ppl1.tile([P, RT], F32, tag="psum_l1")
                      for ct in range(CT):
                          kt_t = plkt.tile([P, R], BF16, tag="lkt_t")
                          nc.sync.dma_start(
                              out=kt_t[:], in_=KT_hbm[ct * P:(ct + 1) * P, :])
                          m1 = plm.tile([P, R], BF16, tag="m1")
                          nc.vector.tensor_mul(m1[:], kt_t[:], u5_rep[:])
                          e1 = plm.tile([P, R], BF16, tag="e1")
                          nc.scalar.activation(out=e1[:], in_=m1[:], func=Exp,
                                               scale=v4v[:, ct, 0:1])
                          for rb in range(RT):
                              nc.tensor.matmul(
                                  psum_l1[:, rb:rb + 1],
                                  lhsT=e1[:, rb * P:(rb + 1) * P],
                                  rhs=ones_bf[:],
                                  start=(ct == 0 and rb == 0),
                                  stop=(ct == CT - 1 and rb == RT - 1),
                                  skip_group_check=True)
                          m2 = plm.tile([P, R], BF16, tag="m2")
                          nc.vector.tensor_mul(m2[:], kt_t[:], vp5_rep[:])
                          e2 = plm.tile([P, R], BF16, tag="e2")
                          nc.scalar.activation(out=e2[:], in_=m2[:], func=Exp,
                                               scale=v5v[:, ct, 1:2],
                                               accum_out=cse2_f[:, ct:ct + 1])

                  if loss_stop == 3:
                      dbg = pp.tile([P, 1], F32, tag="dbg")
                      nc.vector.tensor_reduce(dbg[:], cse2_f[:], axis=mybir.AxisListType.X, op=ADD)
                      nc.sync.dma_start(out=out_loss[:], in_=dbg[0:1, 0:1])
                  if loss_stop == 0:
                      # loss1 local total: sum_p sum_rb (log(sum e1) - pd1)
                      lse1 = pls.tile([P, RT], F32, tag="lse1")
                      nc.scalar.activation(out=lse1[:], in_=psum_l1[:], func=Log)
                      d1 = pls.tile([P, RT], F32, tag="d1")
                      nc.vector.tensor_sub(d1[:], lse1[:], pd1[:])
                      pack2 = pls.tile([P, 2], F32, tag="pack2")
                      nc.vector.tensor_reduce(pack2[:, 0:1], d1[:],
                                              axis=mybir.AxisListType.X, op=ADD)
                      nc.vector.tensor_reduce(pack2[:, 1:2], pd2[:],
                                              axis=mybir.AxisListType.X, op=ADD)
                      psum_sc = pplb.tile([1, 2], F32, tag="psum_sc")
                      nc.tensor.matmul(psum_sc[:], lhsT=ones_f[:], rhs=pack2[:],
                                       start=True, stop=True)

                      # second AR: cse2 partials + the two scalars
                      stage2 = pls.tile([P, CT + 2], F32, tag="stage2")
                      nc.vector.memset(stage2[:], 0.0)
                      nc.vector.tensor_copy(stage2[:, 0:CT], cse2_f[:])
                      nc.scalar.copy(out=stage2[0:1, CT:CT + 2], in_=psum_sc[:])
                      g2 = pls.tile([P, CT + 2], F32, tag="g2")
                      if NCORES == 1:
                          nc.gpsimd.dma_start(out=g2[:], in_=stage2[:])
                      else:
                          nc.gpsimd.dma_start(out=ar2_in[:], in_=stage2[:])
                          nc.gpsimd.collective_compute(
                              "AllReduce", ADD, replica_groups=RG,
                              ins=[ar2_in[:]], outs=[ar2_out[:]])
                          nc.gpsimd.dma_start(out=g2[:], in_=ar2_out[:])

                      lse2 = pls.tile([P, CT], F32, tag="lse2")
                      nc.scalar.activation(out=lse2[:], in_=g2[:, 0:CT], func=Log)
                      l2s = pls.tile([P, 1], F32, tag="l2s")
                      nc.vector.tensor_reduce(l2s[:], lse2[:],
                                              axis=mybir.AxisListType.X, op=ADD)
                      psum_fs = pplb.tile([1, 1], F32, tag="psum_fs")
                      nc.tensor.matmul(psum_fs[:], lhsT=ones_f[:], rhs=l2s[:],
                                       start=True, stop=True)
                      fin = pls.tile([1, 1], F32, tag="fin")
                      nc.scalar.copy(out=fin[:], in_=psum_fs[:])
                      nc.vector.tensor_add(fin[:], fin[:], g2[0:1, CT:CT + 1])
                      nc.vector.tensor_sub(fin[:], fin[:], g2[0:1, CT + 1:CT + 2])
                      nc.scalar.mul(out=fin[:], in_=fin[:], mul=1.0 / (2 * B))
                      nc.sync.dma_start(out=out_loss[:], in_=fin[:])

    nc.compile()
    return nc


_NC_CACHE = None


def _get_nc():
    global _NC_CACHE
    if _NC_CACHE is None:
        _NC_CACHE = _build_bass()
    return _NC_CACHE


def make_in_maps(all_image_features, all_text_features):
    img = np.asarray(all_image_features, np.float32)
    txt = np.asarray(all_text_features, np.float32)

    img_bf = img.astype(ml_dtypes.bfloat16)
    txt_bf = txt.astype(ml_dtypes.bfloat16)
    # [d, x] -> [dlo, dhi, x] with d = dhi*128 + dlo
    imgT = np.ascontiguousarray(
        img_bf.T.reshape(2, P, B).transpose(1, 0, 2))
    txtT = np.ascontiguousarray(
        txt_bf.T.reshape(2, P, B).transpose(1, 0, 2))

    # host-side K diagonal (consistent with bf16 GEMM inputs, fp32 exp)
    sdiag = np.einsum("bd,bd->b",
                      img_bf.astype(np.float32), txt_bf.astype(np.float32))
    kdiag = np.exp(INV_REG * sdiag - INV_REG).astype(np.float32)

    in_maps = []
    for c in range(NCORES):
        rows = slice(c * R, (c + 1) * R)
        kd = np.ascontiguousarray(
            kdiag[rows].reshape(RT, P).T).astype(np.float32)  # [p, rb]
        # dsel[p, rb, cb] = 1 iff cb == c*RT + rb  (same for all p)
        dsel = np.zeros((P, RT, CT), np.float32)
        for rb in range(RT):
            dsel[:, rb, c * RT + rb] = 1.0
        in_maps.append({
            "imgT": np.ascontiguousarray(imgT[:, :, rows]),
            "txtT": txtT,
            "kdiag": kd,
            "dsel": dsel,
        })
    return in_maps


def kernel(all_image_features, all_text_features, logit_scale, labels):
    in_maps = make_in_maps(all_image_features, all_text_features)
    nc = _get_nc()
    res = bass_utils.run_bass_kernel_spmd(
        nc, in_maps, core_ids=list(range(NCORES)))
    loss = res.results[0]["out_loss"][0, 0]
    return np.asarray(loss, dtype=np.float32)



# revision 6
# speedup vs baseline: 1.3599x; 1.3599x over previous
"""Entropic Sinkhorn loss kernel for Trainium2 (8 NeuronCores, SPMD).

Math (reference): loss = (sinkhorn(1 - img@txt.T) + sinkhorn((1 - img@txt.T).T)) / 2
with K = exp(-M/0.01) = exp(100*S - 100), S = img@txt.T, 5 Sinkhorn iterations,
then P = u * K * v, loss_half = -mean(log_softmax(P)[i, i]).

Device algorithm (per core, rows sharded R = B/8):
  build B1: S row-shard GEMM (bf16) -> K = exp(100S-100) (bf16) resident in
            SBUF [128, RB, B]; fold in the first col-matvec KtU_1 = K^T (1/n)
            (lhsT = K blocks, rhs = const) -> AR -> v_1.
  build B2: swapped-operand GEMM (txt as lhsT) -> exp -> KT tiles [128c, R]
            written to HBM (row sweeps + loss stream them back, prefetched).
  pass t (1..5), both chains packed as width-2 vectors:
    row sweep (rhs-oriented): out[2, R] += cv[t-1][128c,2]^T @ KT_tile[128c, R]
      over CB c-blocks; tiny [2,128] PE transposes -> canonical ru[t][128r, RB, 2]
      = (u_t = 1/(K v_t), v'_t = b/(K u'_{t-1})).
    col sweep (lhsT-oriented): psum[128c, CB, 2] += K_blk[128r,128c]^T @ ru[t][128r,2]
      -> 64KB AllReduce -> cv[t][128c, CB, 2] = (v_{t+1} = b/AR, u'_t = 1/AR).
  loss: stream KT once; e1 = exp(v5_c * KT[c,r] * u5_r) col-summed on PE
        (row-LSE of P1, local rows); e2 = exp(u'5_c * KT[c,r] * v'5_r)
        row-summed via ACT accum (row-LSE of P2, AR'd); diagonals from host
        kdiag + dsel mask-selects; final scalar assembled on every core.
"""

import numpy as np
import ml_dtypes

import concourse.bass as bass
import concourse.bacc as bacc
import concourse.tile as tile
import concourse.mybir as mybir
from concourse import bass_utils
from concourse.masks import make_identity

B = 8192
D = 256
P = 128
NCORES = 8
R = B // NCORES            # rows per core
RB = R // P                # row blocks per core
CB = B // P                # col blocks
N_ITER = 5
INV_REG = 100.0
BVAL = 1.0 / B

BF16 = mybir.dt.bfloat16
F32 = mybir.dt.float32
Exp = mybir.ActivationFunctionType.Exp
Log = mybir.ActivationFunctionType.Ln
ADD = mybir.AluOpType.add
X = mybir.AxisListType.X


def _set_dims(b, ncores):
    global B, NCORES, R, RB, CB, BVAL
    if ncores is not None:
        NCORES = ncores
    if b is not None:
        B = b
    R = B // NCORES
    RB = R // P
    CB = B // P
    BVAL = 1.0 / B


def _build_bass(b=None, phases=3, ncores=None):
    _set_dims(b, ncores)
    H = (R + 511) // 512          # 512-wide chunks of local rows
    W = min(512, R)               # row-chunk width
    nc = bacc.Bacc("TRN2", target_bir_lowering=False, debug=False,
                   num_devices=NCORES)

    imgT = nc.dram_tensor("imgT", [P, 2, R], BF16, kind="ExternalInput")
    txtT = nc.dram_tensor("txtT", [P, 2, B], BF16, kind="ExternalInput")
    kdiag_in = nc.dram_tensor("kdiag", [P, RB], F32, kind="ExternalInput")
    dsel_in = nc.dram_tensor("dsel", [P, RB, CB], F32, kind="ExternalInput")
    out_loss = nc.dram_tensor("out_loss", [1, 1], F32, kind="ExternalOutput")

    KT_hbm = nc.dram_tensor("KT_hbm", [B, R], BF16)
    RG = [list(range(NCORES))]
    ar0_in = nc.dram_tensor("ar0_in", [P, CB], F32)
    ar0_out = nc.dram_tensor("ar0_out", [P, CB], F32, addr_space="Shared")
    ar_in = [nc.dram_tensor(f"ar_in{t}", [P, 2 * CB], F32)
             for t in range(1, N_ITER + 1)]
    ar_out = [nc.dram_tensor(f"ar_out{t}", [P, 2 * CB], F32,
                             addr_space="Shared")
              for t in range(1, N_ITER + 1)]
    ar2_in = nc.dram_tensor("ar2_in", [P, 72], F32)
    ar2_out = nc.dram_tensor("ar2_out", [P, 72], F32, addr_space="Shared")

    def allreduce(src_ap, dst_ap, din, dout):
        if NCORES == 1:
            nc.gpsimd.dma_start(out=dst_ap, in_=src_ap)
            return
        nc.gpsimd.dma_start(out=din[:], in_=src_ap)
        nc.gpsimd.collective_compute(
            "AllReduce", ADD, replica_groups=RG, ins=[din[:]], outs=[dout[:]])
        nc.gpsimd.dma_start(out=dst_ap, in_=dout[:])

    with tile.TileContext(nc) as tc:
        with tc.tile_pool(name="persist", bufs=1) as pp, \
             tc.tile_pool(name="ktstream", bufs=8) as pkt:
            ident_bf = pp.tile([P, P], BF16, tag="ident_bf")
            make_identity(nc, ident_bf[:])
            ident_f = pp.tile([P, P], F32, tag="ident_f")
            make_identity(nc, ident_f[:])
            ones_bf = pp.tile([P, 1], BF16, tag="ones_bf")
            nc.vector.memset(ones_bf[:], 1.0)
            ones_f = pp.tile([P, 1], F32, tag="ones_f")
            nc.vector.memset(ones_f[:], 1.0)
            ones_row = pp.tile([1, P], BF16, tag="ones_row")
            nc.vector.memset(ones_row[:], 1.0)
            invn_bf = pp.tile([P, 1], BF16, tag="invn")
            nc.vector.memset(invn_bf[:], BVAL)
            biasm100 = pp.tile([P, 1], F32, tag="biasm100")
            nc.vector.memset(biasm100[:], -INV_REG)
            kdiag_f = pp.tile([P, RB], F32, tag="kdiag")
            nc.sync.dma_start(out=kdiag_f[:], in_=kdiag_in[:])
            dsel_f = pp.tile([P, RB, CB], F32, tag="dsel")
            nc.sync.dma_start(out=dsel_f[:], in_=dsel_in[:])

            # resident K, row-major: [p, rb, c]
            K_sb = pp.tile([P, RB, B], BF16, tag="K_sb")
            # canonical vectors (flat [P, 2*n], packed (idx, chain)):
            # cv[t] = (v_{t+1}, u'_t) on col blocks; ru[t] = (u_t, v'_t) on rows
            cv_f = [pp.tile([P, 2 * CB], F32, tag=f"cvf{t}", name=f"cvf{t}")
                    for t in range(N_ITER + 1)]
            cv_bf = [pp.tile([P, 2 * CB], BF16, tag=f"cvbf{t}", name=f"cvbf{t}")
                     for t in range(N_ITER + 1)]
            ru_f = [pp.tile([P, 2 * RB], F32, tag=f"ruf{t}", name=f"ruf{t}")
                    for t in range(N_ITER + 1)]
            ru_bf = [pp.tile([P, 2 * RB], BF16, tag=f"rubf{t}", name=f"rubf{t}")
                     for t in range(N_ITER + 1)]
            cvv_f = [ap[:].rearrange("p (c v) -> p c v", v=2) for ap in cv_f]
            cvv_bf = [ap[:].rearrange("p (c v) -> p c v", v=2) for ap in cv_bf]
            ruv_f = [ap[:].rearrange("p (r v) -> p r v", v=2) for ap in ru_f]
            ruv_bf = [ap[:].rearrange("p (r v) -> p r v", v=2) for ap in ru_bf]
            cse2 = pp.tile([P, CB], F32, tag="cse2")
            u_rep = pp.tile([P, R], BF16, tag="u_rep")
            vp_rep = pp.tile([P, R], BF16, tag="vp_rep")

            # ---------------- build ----------------
            with tc.tile_pool(name="bfeat", bufs=1) as pf:
                imgT_sb = pf.tile([P, 2, R], BF16, tag="imgT")
                txtT_sb = pf.tile([P, 2, B], BF16, tag="txtT")
                nc.sync.dma_start(out=imgT_sb[:], in_=imgT[:])
                nc.sync.dma_start(out=txtT_sb[:], in_=txtT[:])

                # B1: K = exp(100 S - 100), resident; fold KtU_1 accumulation
                CH = max(B // 1024, 1)      # 1024-wide col chunks
                CW = min(1024, B)
                with tc.tile_pool(name="bps", bufs=3, space="PSUM") as pps, \
                     tc.tile_pool(name="bp0", bufs=1, space="PSUM") as pp0:
                    psum0 = pp0.tile([P, CB], F32, tag="psum0")
                    for rb in range(RB):
                        for ch in range(CH):
                            psum_s = pps.tile([P, CW], F32, tag="psum_s")
                            for sub in range(CW // 512):
                                sl = slice(ch * CW + sub * 512,
                                           ch * CW + (sub + 1) * 512)
                                psl = slice(sub * 512, (sub + 1) * 512)
                                for dhi in range(2):
                                    nc.tensor.matmul(
                                        psum_s[:, psl],
                                        lhsT=imgT_sb[:, dhi,
                                                     rb * P:(rb + 1) * P],
                                        rhs=txtT_sb[:, dhi, sl],
                                        start=(dhi == 0), stop=(dhi == 1),
                                        skip_group_check=True)
                            nc.scalar.activation(
                                out=K_sb[:, rb, ch * CW:(ch + 1) * CW],
                                in_=psum_s[:], func=Exp,
                                scale=INV_REG, bias=biasm100[:])
                            for cbb in range(CW // P):
                                cb = ch * (CW // P) + cbb
                                nc.tensor.matmul(
                                    psum0[:, cb:cb + 1],
                                    lhsT=K_sb[:, rb, cb * P:(cb + 1) * P],
                                    rhs=invn_bf[:],
                                    start=(rb == 0 and cb == 0),
                                    stop=(rb == RB - 1 and cb == CB - 1),
                                    skip_group_check=True)
                    # AR0 -> v_1 = b/KtU_1 ; u'_0 = 1/n
                    cs0 = pf.tile([P, CB], F32, tag="cs0")
                    nc.vector.tensor_copy(cs0[:], psum0[:])
                g0 = pf.tile([P, CB], F32, tag="g0")
                allreduce(cs0[:], g0[:], ar0_in, ar0_out)
                nc.vector.memset(cv_f[0][:], BVAL)
                nc.vector.reciprocal(cvv_f[0][:, :, 0], g0[:])
                nc.vector.tensor_scalar_mul(
                    cvv_f[0][:, :, 0], cvv_f[0][:, :, 0], BVAL)
                nc.vector.tensor_copy(cv_bf[0][:], cv_f[0][:])

                # B2: KT tiles via swapped GEMM -> HBM
                with tc.tile_pool(name="bkt", bufs=3) as pkts, \
                     tc.tile_pool(name="bpk", bufs=2, space="PSUM") as ppk:
                    for cb in range(CB):
                        psum_kt = ppk.tile([P, R], F32, tag="psum_kt")
                        for dhi in range(2):
                            for rt in range(H):
                                nc.tensor.matmul(
                                    psum_kt[:, rt * W:(rt + 1) * W],
                                    lhsT=txtT_sb[:, dhi, cb * P:(cb + 1) * P],
                                    rhs=imgT_sb[:, dhi, rt * W:(rt + 1) * W],
                                    start=(dhi == 0), stop=(dhi == 1),
                                    skip_group_check=True)
                        ktst = pkts.tile([P, R], BF16, tag="ktst")
                        nc.scalar.activation(out=ktst[:], in_=psum_kt[:],
                                             func=Exp, scale=INV_REG,
                                             bias=biasm100[:])
                        nc.sync.dma_start(out=KT_hbm[cb * P:(cb + 1) * P, :],
                                          in_=ktst[:])

            if phases < 2:
                dbg = pp.tile([P, 1], F32, tag="dbg")
                nc.vector.tensor_reduce(dbg[:], cv_f[0][:], axis=X, op=ADD)
                nc.sync.dma_start(out=out_loss[:], in_=dbg[0:1, 0:1])

            # ---------------- sinkhorn passes ----------------
            if phases >= 2:
              with tc.tile_pool(name="sps", bufs=1) as psv, \
                   tc.tile_pool(name="spr", bufs=1, space="PSUM") as ppr, \
                   tc.tile_pool(name="spt", bufs=1, space="PSUM") as ppt, \
                   tc.tile_pool(name="spc", bufs=1, space="PSUM") as ppc:
                for t in range(1, N_ITER + 1):
                    # row sweep: out[2, R] over CB c-blocks of KT
                    psum_r = ppr.tile([2, R], F32, tag="psum_r")
                    for cb in range(CB):
                        kt_t = pkt.tile([P, R], BF16, tag="kt_t")
                        nc.sync.dma_start(
                            out=kt_t[:], in_=KT_hbm[cb * P:(cb + 1) * P, :])
                        for h in range(H):
                            nc.tensor.matmul(
                                psum_r[:, h * W:(h + 1) * W],
                                lhsT=cvv_bf[t - 1][:, cb, :],
                                rhs=kt_t[:, h * W:(h + 1) * W],
                                start=(cb == 0), stop=(cb == CB - 1),
                                skip_group_check=True)
                    rsums = psv.tile([2, R], F32, tag="rsums")
                    nc.vector.tensor_copy(rsums[:], psum_r[:])
                    psum_t = ppt.tile([P, 2 * RB], F32, tag="psum_t")
                    for rb in range(RB):
                        nc.tensor.transpose(
                            psum_t[:, 2 * rb:2 * rb + 2],
                            rsums[0:2, rb * P:(rb + 1) * P],
                            ident_f[0:2, 0:2])
                    nc.vector.reciprocal(ru_f[t][:], psum_t[:])
                    nc.vector.tensor_scalar_mul(
                        ruv_f[t][:, :, 1], ruv_f[t][:, :, 1], BVAL)
                    nc.vector.tensor_copy(ru_bf[t][:], ru_f[t][:])

                    # col sweep: psum[128c, CB, 2] from resident K
                    psum_c = ppc.tile([P, 2 * CB], F32, tag="psum_c")
                    psum_cv = psum_c[:].rearrange("p (c v) -> p c v", v=2)
                    for rb in range(RB):
                        for cb in range(CB):
                            nc.tensor.matmul(
                                psum_cv[:, cb, :],
                                lhsT=K_sb[:, rb, cb * P:(cb + 1) * P],
                                rhs=ruv_bf[t][:, rb, :],
                                start=(rb == 0 and cb == 0),
                                stop=(rb == RB - 1 and cb == CB - 1),
                                skip_group_check=True)
                    cs = psv.tile([P, 2 * CB], F32, tag="cs")
                    nc.vector.tensor_copy(cs[:], psum_c[:])
                    gg = psv.tile([P, 2 * CB], F32, tag="gg")
                    allreduce(cs[:], gg[:], ar_in[t - 1], ar_out[t - 1])
                    nc.vector.reciprocal(cv_f[t][:], gg[:])
                    nc.vector.tensor_scalar_mul(
                        cvv_f[t][:, :, 0], cvv_f[t][:, :, 0], BVAL)
                    nc.vector.tensor_copy(cv_bf[t][:], cv_f[t][:])

            if phases == 2:
                dbg = pp.tile([P, 1], F32, tag="dbg")
                nc.vector.tensor_reduce(dbg[:], cv_f[N_ITER][:], axis=X,
                                        op=ADD)
                nc.sync.dma_start(out=out_loss[:], in_=dbg[0:1, 0:1])

            # ---------------- loss ----------------
            if phases >= 3:
              with tc.tile_pool(name="lsm", bufs=1) as pls, \
                   tc.tile_pool(name="lm", bufs=3) as plm, \
                   tc.tile_pool(name="le", bufs=3) as ple, \
                   tc.tile_pool(name="lpf", bufs=1, space="PSUM") as pplf, \
                   tc.tile_pool(name="lpb", bufs=1, space="PSUM") as pplb, \
                   tc.tile_pool(name="lp1", bufs=1, space="PSUM") as ppl1:
                # u5 / v'5 replicated along partitions: rep[p, i] = x[i]
                for vec, rep in ((0, u_rep), (1, vp_rep)):
                    psum_fl = pplf.tile([1, R], F32, tag="psum_fl",
                                        name=f"psum_fl{vec}")
                    for rb in range(RB):
                        nc.tensor.matmul(
                            psum_fl[0:1, rb * P:(rb + 1) * P],
                            lhsT=ruv_bf[N_ITER][:, rb, vec:vec + 1],
                            rhs=ident_bf[:],
                            start=True, stop=True, skip_group_check=True)
                    flat = pls.tile([1, R], BF16, tag=f"flat{vec}",
                                    name=f"flat{vec}")
                    nc.vector.tensor_copy(flat[:], psum_fl[:])
                    psum_bc = pplb.tile([P, R], F32, tag="psum_bc",
                                        name=f"psum_bc{vec}")
                    for h in range(H):
                        nc.tensor.matmul(
                            psum_bc[:, h * W:(h + 1) * W],
                            lhsT=ones_row[0:1, 0:P],
                            rhs=flat[0:1, h * W:(h + 1) * W],
                            start=True, stop=True, skip_group_check=True)
                    nc.vector.tensor_copy(rep[:], psum_bc[:])

                # main stream: e1 col-sums (PE), e2 row-sums (ACT accum)
                psum_l1 = ppl1.tile([1, R], F32, tag="psum_l1")
                for cb in range(CB):
                    kt_t = pkt.tile([P, R], BF16, tag="kt_t")
                    nc.sync.dma_start(
                        out=kt_t[:], in_=KT_hbm[cb * P:(cb + 1) * P, :])
                    m1 = plm.tile([P, R], BF16, tag="m1")
                    nc.vector.tensor_mul(m1[:], kt_t[:], u_rep[:])
                    e1 = ple.tile([P, R], BF16, tag="e1")
                    nc.scalar.activation(out=e1[:], in_=m1[:], func=Exp,
                                         scale=cvv_f[4][:, cb, 0:1])
                    for h in range(H):
                        nc.tensor.matmul(
                            psum_l1[0:1, h * W:(h + 1) * W],
                            lhsT=ones_bf[:],
                            rhs=e1[:, h * W:(h + 1) * W],
                            start=(cb == 0), stop=(cb == CB - 1),
                            skip_group_check=True)
                    m2 = plm.tile([P, R], BF16, tag="m2")
                    nc.vector.tensor_mul(m2[:], kt_t[:], vp_rep[:])
                    e2 = ple.tile([P, R], BF16, tag="e2")
                    nc.scalar.activation(out=e2[:], in_=m2[:], func=Exp,
                                         scale=cvv_f[N_ITER][:, cb, 1:2],
                                         accum_out=cse2[:, cb:cb + 1])

                # row-LSE of P1 (local) and diagonal terms
                lse1 = pls.tile([1, R], F32, tag="lse1")
                l1sum = pls.tile([1, 1], F32, tag="l1sum")
                nc.scalar.activation(out=lse1[:], in_=psum_l1[:], func=Log,
                                     accum_out=l1sum[:])
                v5loc = pls.tile([P, RB], F32, tag="v5loc")
                uploc = pls.tile([P, RB], F32, tag="uploc")
                selscr = pls.tile([P, CB], F32, tag="selscr")
                for rb in range(RB):
                    nc.vector.tensor_mul(selscr[:], cvv_f[4][:, :, 0],
                                         dsel_f[:, rb, :])
                    nc.vector.tensor_reduce(v5loc[:, rb:rb + 1], selscr[:],
                                            axis=X, op=ADD)
                    nc.vector.tensor_mul(selscr[:], cvv_f[N_ITER][:, :, 1],
                                         dsel_f[:, rb, :])
                    nc.vector.tensor_reduce(uploc[:, rb:rb + 1], selscr[:],
                                            axis=X, op=ADD)
                pd1 = pls.tile([P, RB], F32, tag="pd1")
                pd2 = pls.tile([P, RB], F32, tag="pd2")
                nc.vector.tensor_mul(pd1[:], ruv_f[N_ITER][:, :, 0],
                                     kdiag_f[:])
                nc.vector.tensor_mul(pd1[:], pd1[:], v5loc[:])
                nc.vector.tensor_mul(pd2[:], ruv_f[N_ITER][:, :, 1],
                                     kdiag_f[:])
                nc.vector.tensor_mul(pd2[:], pd2[:], uploc[:])
                pack = pls.tile([P, 2], F32, tag="pack")
                nc.vector.tensor_reduce(pack[:, 0:1], pd1[:], axis=X, op=ADD)
                nc.vector.tensor_reduce(pack[:, 1:2], pd2[:], axis=X, op=ADD)
                psum_sc = pplb.tile([1, 2], F32, tag="psum_sc")
                nc.tensor.matmul(psum_sc[:], lhsT=ones_f[:], rhs=pack[:],
                                 start=True, stop=True, skip_group_check=True)

                # final AR: cse2 partials + the two local scalars
                stage2 = pls.tile([P, 72], F32, tag="stage2")
                nc.vector.memset(stage2[:], 0.0)
                nc.vector.tensor_copy(stage2[:, 0:CB], cse2[:])
                nc.vector.tensor_sub(stage2[0:1, CB:CB + 1], l1sum[:],
                                     psum_sc[0:1, 0:1])
                nc.vector.tensor_copy(stage2[0:1, CB + 1:CB + 2],
                                      psum_sc[0:1, 1:2])
                g2 = pls.tile([P, 72], F32, tag="g2")
                allreduce(stage2[:], g2[:], ar2_in, ar2_out)
                lse2 = pls.tile([P, CB], F32, tag="lse2")
                l2s = pls.tile([P, 1], F32, tag="l2s")
                nc.scalar.activation(out=lse2[:], in_=g2[:, 0:CB], func=Log,
                                     accum_out=l2s[:])
                psum_fs = pplb.tile([1, 1], F32, tag="psum_fs")
                nc.tensor.matmul(psum_fs[:], lhsT=ones_f[:], rhs=l2s[:],
                                 start=True, stop=True, skip_group_check=True)
                fin = pls.tile([1, 1], F32, tag="fin")
                nc.scalar.copy(out=fin[:], in_=psum_fs[:])
                nc.vector.tensor_add(fin[:], fin[:], g2[0:1, CB:CB + 1])
                nc.vector.tensor_sub(fin[:], fin[:], g2[0:1, CB + 1:CB + 2])
                nc.scalar.mul(out=fin[:], in_=fin[:], mul=1.0 / (2 * B))
                nc.sync.dma_start(out=out_loss[:], in_=fin[:])

    nc.compile()
    return nc


_NC_CACHE = None


def _get_nc():
    global _NC_CACHE
    if _NC_CACHE is None:
        _NC_CACHE = _build_bass()
    return _NC_CACHE


def make_in_maps(all_image_features, all_text_features):
    img = np.asarray(all_image_features, np.float32)
    txt = np.asarray(all_text_features, np.float32)

    img_bf = img.astype(ml_dtypes.bfloat16)
    txt_bf = txt.astype(ml_dtypes.bfloat16)
    # [d, x] -> [dlo, dhi, x] with d = dhi*128 + dlo
    imgT = np.ascontiguousarray(img_bf.T.reshape(2, P, B).transpose(1, 0, 2))
    txtT = np.ascontiguousarray(txt_bf.T.reshape(2, P, B).transpose(1, 0, 2))

    # host-side K diagonal (consistent with bf16 GEMM inputs, fp32 exp)
    sdiag = np.einsum("bd,bd->b",
                      img_bf.astype(np.float32), txt_bf.astype(np.float32))
    kdiag = np.exp(INV_REG * sdiag - INV_REG).astype(np.float32)

    in_maps = []
    for c in range(NCORES):
        rows = slice(c * R, (c + 1) * R)
        kd = np.ascontiguousarray(
            kdiag[rows].reshape(RB, P).T).astype(np.float32)  # [p, rb]
        # dsel[p, rb, cb] = 1 iff cb == c*RB + rb  (same for all p)
        dsel = np.zeros((P, RB, CB), np.float32)
        for rb in range(RB):
            dsel[:, rb, c * RB + rb] = 1.0
        in_maps.append({
            "imgT": np.ascontiguousarray(imgT[:, :, rows]),
            "txtT": txtT,
            "kdiag": kd,
            "dsel": dsel,
        })
    return in_maps


def kernel(all_image_features, all_text_features, logit_scale, labels):
    in_maps = make_in_maps(all_image_features, all_text_features)
    nc = _get_nc()
    res = bass_utils.run_bass_kernel_spmd(
        nc, in_maps, core_ids=list(range(NCORES)))
    loss = res.results[0]["out_loss"][0, 0]
    return np.asarray(loss, dtype=np.float32)
